# revision 1
# baseline (speedup 1.0000x reference)
"""Contrastive loss kernel for 8 Trainium2 NeuronCores.

Math (reference):
    s = cosine similarity matrix of x [8192, 256]
    d_i = sum_j exp(s_ij * m_ij / tau)   (m zeroes the diagonal -> diag term = 1)
    v_i = s[i, i^1]                      (adjacent-row positive pairs)
    loss = mean(log d_i - v_i / tau)

Distribution: row-shard across 8 cores. Host normalizes rows of x (0.01% of
the FLOPs), transposes to xnT [256, 8192], and per core ROTATES the columns
so each core's own 1024 rows sit at columns 0..1023.  That makes the SPMD
program position-independent: the diagonal/pair blocks are always at a fixed
(compile-time) location, while row sums are invariant to column order.

Device (per core, identical program):
    - big matmul  s_tile = xnT[:, m*128:...].T @ xnT   (bf16 in, fp32 PSUM;
      fp32r measured ~5x slower on HW despite the cost model)
    - fused exp+row-sum on the scalar engine (accum_out), reading PSUM
      supertiles [128, 2048], writing bf16 exp values to SBUF
    - exp(s_ii/tau) and exp(v_i/tau) extracted from the s=0 exp tile with
      mask-multiply-reduce on the vector engine (SBUF only)
Host combines: d_i = rowsum - exp_diag + 1; loss = mean(log d - log exp_v).
Measured ~78.9us/core one-shot (incl. 4MB input DMA) via For_i repeat-diff;
PE-bound (PE-only floor 73us: 256 MMs x [213ns stream + ~53ns LDW]).

NOTE on structure: walrus codegen allows at most ONE semaphore wait per
engine instruction, so the program is arranged so every instruction depends
on at most one not-yet-observed engine (warmup touches + observer copies).
"""

import os
import sys

import numpy as np

sys.path.insert(0, "/opt/trn_rl_repo")

import concourse.bass as bass
import concourse.tile as tile
from concourse import mybir
from concourse.bass_utils import run_bass_kernel_spmd

import os as _os_early
TAU = 0.1
N = 8192
D = 256
P = 128
NCORES = 8
ROWS_PER_CORE = N // NCORES          # 1024
M_TILES = ROWS_PER_CORE // P         # 8
SUPER = int(_os_early.environ.get("KERNEL_SUPER", "2048"))  # ACT supertile width
S_TILES = N // SUPER
SUB = SUPER // 512                   # matmuls of N=512 per supertile
CHUNK = 2048                         # input DMA chunk width
C_TILES = N // CHUNK
FP32 = mybir.dt.float32
FP32R = mybir.dt.float32r
# matmul input dtype: "fp32r" or "bf16"
import os as _os
MM_DT = _os.environ.get("KERNEL_MM_DT", "bf16")
EO_DT = _os.environ.get("KERNEL_EO_DT", "bf16")   # exp-output dtype knob

_CACHE = {}


def build_nc(repeat=1):
    mmdt = FP32R if MM_DT == "fp32r" else mybir.dt.bfloat16
    xtdt = FP32 if MM_DT == "fp32r" else mybir.dt.bfloat16
    nc = bass.Bass(trn_type="TRN2")
    xt_d = nc.declare_dram_parameter("xt", [2, P, N], xtdt, isOutput=False)
    eodt_d = FP32 if EO_DT == "fp32" else mybir.dt.bfloat16
    eye_d = nc.declare_dram_parameter("eye", [P, P], eodt_d, isOutput=False)
    pm_d = nc.declare_dram_parameter("pm", [P, P], eodt_d, isOutput=False)
    acc_d = nc.declare_dram_parameter("acc", [P, M_TILES * S_TILES], FP32, isOutput=True)
    dv_d = nc.declare_dram_parameter("dv", [P, 2 * M_TILES], FP32, isOutput=True)

    with tile.TileContext(nc) as tc:
        with (
            tc.tile_pool(name="big", bufs=2) as big,
            tc.tile_pool(name="small", bufs=1) as small,
            tc.tile_pool(name="scratch", bufs=4) as sc,
            tc.tile_pool(name="psum", bufs=int(_os.environ.get("KERNEL_PSUM_BUFS", "2")), space="PSUM") as pp,
        ):
            eodt = FP32 if EO_DT == "fp32" else mybir.dt.bfloat16
            eye = small.tile([P, P], eodt, tag="eye")
            pm = small.tile([P, P], eodt, tag="pm")
            acc_sb = small.tile([P, M_TILES * S_TILES], FP32, tag="accsb")
            dv_sb = small.tile([P, 2 * M_TILES], FP32, tag="dvsb")

            nc.sync.dma_start(out=eye, in_=eye_d[:, :])
            nc.sync.dma_start(out=pm, in_=pm_d[:, :])
            # Warmup: make DVE/ACT observe the mask DMAs (and load the Exp
            # table) before the main loop, so steady-state instructions carry
            # a single sem wait (codegen limit) and the ~2.7us ACT table load
            # happens off the critical path.
            warm_v = small.tile([P, 1], FP32, tag="warm_v")
            warm_v2 = small.tile([P, 1], FP32, tag="warm_v2")
            warm_a = small.tile([P, P], FP32, tag="warm_a")
            warm_s = small.tile([P, 1], FP32, tag="warm_s")
            nc.vector.reduce_sum(warm_v, eye, axis=mybir.AxisListType.X)
            nc.vector.reduce_sum(warm_v2, pm, axis=mybir.AxisListType.X)
            nc.scalar.activation(out=warm_a, in_=pm,
                                 func=mybir.ActivationFunctionType.Exp,
                                 scale=1.0, accum_out=warm_s)

            import contextlib
            loop_ctx = (tc.For_i(0, repeat, 1)
                        if repeat > 1 else contextlib.nullcontext())
            with loop_ctx:
                _compute_body(nc, tc, sc, pp, small, big, xt_d, mmdt,
                              eye, pm, acc_sb, dv_sb)

            if _os.environ.get("KERNEL_PE_ONLY", "0") == "1":
                nc.vector.memset(acc_sb, 0.0)
                nc.vector.memset(dv_sb, 0.0)
            nc.sync.dma_start(out=acc_d[:, :], in_=acc_sb)
            nc.sync.dma_start(out=dv_d[:, :], in_=dv_sb)
    _split_multi_waits(nc)
    return nc


def _compute_body(nc, tc, sc, pp, small, big, xt_d, mmdt,
                  eye, pm, acc_sb, dv_sb):
    if _os.environ.get("KERNEL_NULL", "0") == "1":
        nc.vector.memset(acc_sb, 0.0)
        nc.vector.memset(dv_sb, 0.0)
        return
    # x tiles live inside the (bench) loop so input DMA pipelines with the
    # previous iteration's compute; in the one-shot kernel this is just the
    # chunked load.
    xt0 = big.tile([P, N], mmdt, tag="xt0")  # d = 0..127   (k half 0)
    xt1 = big.tile([P, N], mmdt, tag="xt1")  # d = 128..255 (k half 1)
    headopt = _os.environ.get("KERNEL_HEADOPT", "1") == "1"
    if _os.environ.get("KERNEL_CHUNK_DMA", "1") == "1":
        if headopt:
            # split the first chunk pair into 512-wide pieces, k0/k1
            # interleaved, so the first matmul group can start ~1.5us in
            for p_ in range(CHUNK // 512):
                cs = slice(p_ * 512, (p_ + 1) * 512)
                nc.sync.dma_start(out=xt0[:, cs], in_=xt_d[0, :, cs].bitcast(mmdt))
                nc.sync.dma_start(out=xt1[:, cs], in_=xt_d[1, :, cs].bitcast(mmdt))
            first_c = 1
        else:
            first_c = 0
        for c_ in range(first_c, C_TILES):
            cs = slice(c_ * CHUNK, (c_ + 1) * CHUNK)
            nc.sync.dma_start(out=xt0[:, cs], in_=xt_d[0, :, cs].bitcast(mmdt))
            nc.sync.dma_start(out=xt1[:, cs], in_=xt_d[1, :, cs].bitcast(mmdt))
    else:
        nc.sync.dma_start(out=xt0, in_=xt_d[0].bitcast(mmdt))
        nc.sync.dma_start(out=xt1, in_=xt_d[1].bitcast(mmdt))
    if MM_DT == "bf16" and _os.environ.get("KERNEL_HEADOPT", "1") == "1":
        ps_warm = pp.tile([P, SUPER], FP32, tag="super")
        for _w in range(12):
            nc.tensor.matmul(ps_warm[:, 0:P], eye, eye, start=True, stop=True)
    for s in range(S_TILES):
        for m in range(M_TILES):
            lhs0 = xt0[:, m * P:(m + 1) * P]
            lhs1 = xt1[:, m * P:(m + 1) * P]
            if (m == 0 and MM_DT == "bf16" and (s * SUPER) % CHUNK == 0
                    and _os.environ.get("KERNEL_CHUNK_DMA", "1") == "1"):
                # dummy weight loads absorb the chunk-DMA waits on PE
                nc.tensor.ldweights(xt0[:, s * SUPER:s * SUPER + P])
                nc.tensor.ldweights(xt1[:, s * SUPER:s * SUPER + P])
            ps = pp.tile([P, SUPER], FP32, tag="super")
            if _os.environ.get("KERNEL_K_OUTER", "0") == "1":
                # one weight load serves 4 column slices
                for k, (lhs, xt) in enumerate(((lhs0, xt0), (lhs1, xt1))):
                    for sub in range(SUB):
                        cols = slice(s * SUPER + sub * 512,
                                     s * SUPER + (sub + 1) * 512)
                        nc.tensor.matmul(ps[:, sub * 512:(sub + 1) * 512],
                                         lhs, xt[:, cols],
                                         start=(k == 0), stop=(k == 1))
            else:
                for sub in range(SUB):
                    cols = slice(s * SUPER + sub * 512, s * SUPER + (sub + 1) * 512)
                    pslice = ps[:, sub * 512:(sub + 1) * 512]
                    nc.tensor.matmul(pslice, lhs0, xt0[:, cols],
                                     start=True, stop=False)
                    nc.tensor.matmul(pslice, lhs1, xt1[:, cols],
                                     start=False, stop=True)
            if _os.environ.get("KERNEL_PE_ONLY", "0") == "1":
                continue
            # exp + fused row-sum; s=0 exp tiles keep their own slots
            # because DVE reads them (diag/pair extraction).
            eo = sc.tile([P, SUPER], FP32 if EO_DT == "fp32" else mybir.dt.bfloat16,
                         tag="expout0" if s == 0 else "expout")
            nc.scalar.activation(
                out=eo, in_=ps, func=mybir.ActivationFunctionType.Exp,
                scale=1.0 / TAU,
                accum_out=acc_sb[:, m * S_TILES + s:m * S_TILES + s + 1])
            if s == 0:
                gblk = eo[:, m * P:(m + 1) * P]
                tmp = sc.tile([P, P], FP32, tag="gtmp")
                nc.vector.tensor_tensor(
                    out=tmp, in0=gblk, in1=eye, op=mybir.AluOpType.mult)
                nc.vector.reduce_sum(
                    dv_sb[:, m:m + 1], tmp, axis=mybir.AxisListType.X)
                tmp2 = sc.tile([P, P], FP32, tag="gtmp")
                nc.vector.tensor_tensor(
                    out=tmp2, in0=gblk, in1=pm, op=mybir.AluOpType.mult)
                nc.vector.reduce_sum(
                    dv_sb[:, M_TILES + m:M_TILES + m + 1], tmp2,
                    axis=mybir.AxisListType.X)
                # observer: let ACT see the DVE sem so the next
                # s=0 exp's buffer WAR needs no extra wait
                obs = small.tile([P, 1], FP32, tag=f"obs{m}")
                nc.scalar.copy(out=obs,
                               in_=dv_sb[:, M_TILES + m:M_TILES + m + 1])


def _split_multi_waits(nc):
    """walrus codegen accepts at most ONE semaphore wait per engine
    instruction; Tile's wait assignment can bake in several.  Hoist all but
    the last wait of each engine instruction into standalone
    InstEventSemaphore sequencer ops right before it (the same mechanism
    barriers use) — semantics are identical, the engine blocks on the waits
    in order."""
    n_split = 0
    for blk in nc.m.functions[0].blocks:
        new_insts = []
        for inst in blk.instructions:
            si = inst.sync_info
            tname = type(inst).__name__
            if si is not None and len(si.on_wait) > 1 and tname != "InstEventSemaphore":
                waits = list(si.on_wait)
                for j, w in enumerate(waits[:-1]):
                    es = mybir.InstEventSemaphore(
                        name=f"W-split-{inst.name}-{j}")
                    es.engine = inst.engine
                    es.sync_info = mybir.SyncInfo(on_wait=[w], on_update=[])
                    new_insts.append(es)
                    nc.register_instruction(es)
                    n_split += 1
                inst.sync_info = mybir.SyncInfo(
                    on_wait=[waits[-1]], on_update=list(si.on_update))
            new_insts.append(inst)
        blk.instructions[:] = new_insts
    return n_split


def _masks():
    if EO_DT == "fp32":
        mdt = np.float32
    else:
        import ml_dtypes
        mdt = ml_dtypes.bfloat16
    eye = np.eye(P, dtype=mdt)
    pm = np.zeros((P, P), dtype=mdt)
    idx = np.arange(P)
    pm[idx, idx ^ 1] = mdt(1.0)
    return eye, pm


def _prepare_inputs(x):
    x = np.ascontiguousarray(np.asarray(x, dtype=np.float32))
    inv = 1.0 / np.sqrt((x * x).sum(axis=1))
    xn = x * inv[:, None].astype(np.float32)
    if MM_DT == "bf16":
        import ml_dtypes
        xnT = np.ascontiguousarray(xn.T.astype(ml_dtypes.bfloat16))
    else:
        xnT = np.ascontiguousarray(xn.T.astype(np.float32))  # [256, 8192]
    eye, pm = _masks()
    in_maps = []
    for c in range(NCORES):
        rolled = np.roll(xnT, -c * ROWS_PER_CORE, axis=1)
        xt = np.ascontiguousarray(rolled.reshape(2, P, N))
        in_maps.append({"xt": xt, "eye": eye, "pm": pm})
    return in_maps


def _combine(results):
    total = 0.0
    for c in range(NCORES):
        acc = np.asarray(results[c]["acc"], dtype=np.float64)   # [128, 32]
        dv = np.asarray(results[c]["dv"], dtype=np.float64)     # [128, 16]
        rowsum = acc.reshape(P, M_TILES, S_TILES).sum(axis=2)   # [p, m]
        diag_exp = dv[:, :M_TILES]                              # exp(s_ii/tau)
        v_exp = dv[:, M_TILES:]                                 # exp(v_i/tau)
        d = rowsum - diag_exp + 1.0
        total += (np.log(d) - np.log(v_exp)).sum()
    return np.float32(total / N)


def kernel(x, repeat=None):
    if repeat is None:
        repeat = int(os.environ.get("KERNEL_REPEAT", "1"))
    key = f"nc{repeat}"
    if key not in _CACHE:
        _CACHE[key] = build_nc(repeat)
    nc = _CACHE[key]
    in_maps = _prepare_inputs(x)
    trace = bool(int(os.environ.get("KERNEL_TRACE", "0")))
    res = run_bass_kernel_spmd(nc, in_maps, list(range(NCORES)), trace=trace)
    _CACHE["last_results"] = res
    return _combine(res.results)



# revision 6
# speedup vs baseline: 1.9907x; 1.9907x over previous
"""Contrastive loss kernel for 8 Trainium2 NeuronCores.

Math (reference):
    s = cosine similarity matrix of x [8192, 256]
    d_i = sum_j exp(s_ij * m_ij / tau)   (m zeroes the diagonal -> diag term = 1)
    v_i = s[i, i^1]                      (adjacent-row positive pairs)
    loss = mean(log d_i - v_i / tau)

Distribution: row-shard across 8 cores. Host normalizes rows of x (0.01% of
the FLOPs), transposes to xnT [256, 8192], and per core ROTATES the columns
so each core's own 1024 rows sit at columns 0..1023.  That makes the SPMD
program position-independent: the diagonal/pair blocks are always at a fixed
(compile-time) location, while row sums are invariant to column order.

Device (per core, identical program):
    - big matmul  s_tile = xnT[:, m*128:...].T @ xnT   (bf16 in, fp32 PSUM;
      fp32r measured ~5x slower on HW despite the cost model)
    - fused exp+row-sum on the scalar engine (accum_out), reading PSUM
      supertiles [128, 2048], writing bf16 exp values to SBUF
    - exp(s_ii/tau) and exp(v_i/tau) extracted from the s=0 exp tile with
      mask-multiply-reduce on the vector engine (SBUF only)
Host combines: d_i = rowsum - exp_diag + 1; loss = mean(log d - log exp_v).
Measured ~78.9us/core one-shot (incl. 4MB input DMA) via For_i repeat-diff;
PE-bound (PE-only floor 73us: 256 MMs x [213ns stream + ~53ns LDW]).

NOTE on structure: walrus codegen allows at most ONE semaphore wait per
engine instruction, so the program is arranged so every instruction depends
on at most one not-yet-observed engine (warmup touches + observer copies).
"""

import os
import sys

import numpy as np

sys.path.insert(0, "/opt/trn_rl_repo")

import concourse.bass as bass
import concourse.tile as tile
from concourse import mybir
from concourse.bass_utils import run_bass_kernel_spmd

import os as _os_early
TAU = 0.1
N = 8192
D = 256
P = 128
NCORES = 8
ROWS_PER_CORE = N // NCORES          # 1024
M_TILES = ROWS_PER_CORE // P         # 8
SUPER = int(_os_early.environ.get("KERNEL_SUPER", "2048"))  # ACT supertile width
S_TILES = N // SUPER
SUB = SUPER // 512                   # matmuls of N=512 per supertile
CHUNK = 2048                         # input DMA chunk width
C_TILES = N // CHUNK
FP32 = mybir.dt.float32
FP32R = mybir.dt.float32r
FP8 = mybir.dt.float8e4
# matmul input dtype: "fp32r", "bf16", or "fp8" (fp8e4 + DoubleRow: K=256 in
# one pass at 2 rows/cycle -> half the PE streaming of bf16)
import os as _os
MM_DT = _os.environ.get("KERNEL_MM_DT", "fp8")
EO_DT = _os.environ.get("KERNEL_EO_DT", "bf16")   # exp-output dtype knob
FP8_SCALE = 16.0   # host multiplies xn by this before e4m3 rounding

_CACHE = {}


def build_nc(repeat=1):
    mmdt = {"fp32r": FP32R, "bf16": mybir.dt.bfloat16, "fp8": FP8}[MM_DT]
    xtdt = {"fp32r": FP32, "bf16": mybir.dt.bfloat16, "fp8": FP8}[MM_DT]
    nc = bass.Bass(trn_type="TRN2")
    xt_d = nc.declare_dram_parameter("xt", [2, P, N], xtdt, isOutput=False)
    eodt_d = FP32 if EO_DT == "fp32" else mybir.dt.bfloat16
    eye_d = nc.declare_dram_parameter("eye", [P, P], eodt_d, isOutput=False)
    pm_d = nc.declare_dram_parameter("pm", [P, P], eodt_d, isOutput=False)
    acc_d = nc.declare_dram_parameter("acc", [P, M_TILES * S_TILES], FP32, isOutput=True)
    dv_d = nc.declare_dram_parameter("dv", [P, 2 * M_TILES], FP32, isOutput=True)

    with tile.TileContext(nc) as tc:
        with (
            tc.tile_pool(name="big", bufs=2) as big,
            tc.tile_pool(name="small", bufs=1) as small,
            tc.tile_pool(name="scratch", bufs=4) as sc,
            tc.tile_pool(name="psum", bufs=int(_os.environ.get("KERNEL_PSUM_BUFS", "2")), space="PSUM") as pp,
        ):
            eodt = FP32 if EO_DT == "fp32" else mybir.dt.bfloat16
            eye = small.tile([P, P], eodt, tag="eye")
            pm = small.tile([P, P], eodt, tag="pm")
            acc_sb = small.tile([P, M_TILES * S_TILES], FP32, tag="accsb")
            dv_sb = small.tile([P, 2 * M_TILES], FP32, tag="dvsb")

            nc.sync.dma_start(out=eye, in_=eye_d[:, :])
            nc.sync.dma_start(out=pm, in_=pm_d[:, :])
            # Warmup: make DVE/ACT observe the mask DMAs (and load the Exp
            # table) before the main loop, so steady-state instructions carry
            # a single sem wait (codegen limit) and the ~2.7us ACT table load
            # happens off the critical path.
            warm_v = small.tile([P, 1], FP32, tag="warm_v")
            warm_v2 = small.tile([P, 1], FP32, tag="warm_v2")
            warm_a = small.tile([P, P], FP32, tag="warm_a")
            warm_s = small.tile([P, 1], FP32, tag="warm_s")
            nc.vector.reduce_sum(warm_v, eye, axis=mybir.AxisListType.X)
            nc.vector.reduce_sum(warm_v2, pm, axis=mybir.AxisListType.X)
            nc.scalar.activation(out=warm_a, in_=pm,
                                 func=mybir.ActivationFunctionType.Exp,
                                 scale=1.0, accum_out=warm_s)

            import contextlib
            loop_ctx = (tc.For_i(0, repeat, 1)
                        if repeat > 1 else contextlib.nullcontext())
            with loop_ctx:
                _compute_body(nc, tc, sc, pp, small, big, xt_d, mmdt,
                              eye, pm, acc_sb, dv_sb)

            if _os.environ.get("KERNEL_PE_ONLY", "0") == "1":
                nc.vector.memset(acc_sb, 0.0)
                nc.vector.memset(dv_sb, 0.0)
            nc.sync.dma_start(out=acc_d[:, :], in_=acc_sb)
            nc.sync.dma_start(out=dv_d[:, :], in_=dv_sb)
    _split_multi_waits(nc)
    return nc


def _compute_body(nc, tc, sc, pp, small, big, xt_d, mmdt,
                  eye, pm, acc_sb, dv_sb):
    if _os.environ.get("KERNEL_NULL", "0") == "1":
        nc.vector.memset(acc_sb, 0.0)
        nc.vector.memset(dv_sb, 0.0)
        return
    # x tiles live inside the (bench) loop so input DMA pipelines with the
    # previous iteration's compute; in the one-shot kernel this is just the
    # chunked load.
    fp8 = MM_DT == "fp8"
    if fp8:
        # single 3D tile [P, 2, N]: dim1 = k-tile (d half), so DoubleRow
        # matmuls can take (p, 2, free) APs with a regular stride
        xts = big.tile([P, 2, N], mmdt, tag="xts")
        xt0 = xts[:, 0, :]
        xt1 = xts[:, 1, :]
    else:
        xt0 = big.tile([P, N], mmdt, tag="xt0")  # d = 0..127   (k half 0)
        xt1 = big.tile([P, N], mmdt, tag="xt1")  # d = 128..255 (k half 1)
    headopt = _os.environ.get("KERNEL_HEADOPT", "1") == "1"
    if _os.environ.get("KERNEL_CHUNK_DMA", "1") == "1":
        if headopt:
            # split the first chunk pair into 512-wide pieces, k0/k1
            # interleaved, so the first matmul group can start ~1.5us in
            for p_ in range(CHUNK // 512):
                cs = slice(p_ * 512, (p_ + 1) * 512)
                nc.sync.dma_start(out=xt0[:, cs], in_=xt_d[0, :, cs].bitcast(mmdt))
                nc.sync.dma_start(out=xt1[:, cs], in_=xt_d[1, :, cs].bitcast(mmdt))
            first_c = 1
        else:
            first_c = 0
        for c_ in range(first_c, C_TILES):
            cs = slice(c_ * CHUNK, (c_ + 1) * CHUNK)
            nc.sync.dma_start(out=xt0[:, cs], in_=xt_d[0, :, cs].bitcast(mmdt))
            nc.sync.dma_start(out=xt1[:, cs], in_=xt_d[1, :, cs].bitcast(mmdt))
    else:
        nc.sync.dma_start(out=xt0, in_=xt_d[0].bitcast(mmdt))
        nc.sync.dma_start(out=xt1, in_=xt_d[1].bitcast(mmdt))
    if MM_DT in ("bf16", "fp8") and _os.environ.get("KERNEL_HEADOPT", "1") == "1":
        ps_warm = pp.tile([P, SUPER], FP32, tag="super")
        for _w in range(12):
            nc.tensor.matmul(ps_warm[:, 0:P], eye, eye, start=True, stop=True)
    for s in range(S_TILES):
        for m in range(M_TILES):
            if (m == 0 and MM_DT in ("bf16", "fp8") and (s * SUPER) % CHUNK == 0
                    and _os.environ.get("KERNEL_CHUNK_DMA", "1") == "1"):
                # dummy weight loads absorb the chunk-DMA waits on PE
                nc.tensor.ldweights(xt0[:, s * SUPER:s * SUPER + P])
                nc.tensor.ldweights(xt1[:, s * SUPER:s * SUPER + P])
            ps = pp.tile([P, SUPER], FP32, tag="super")
            if fp8:
                lhs = xts[:, 0:2, m * P:(m + 1) * P]
                for sub in range(SUB):
                    cols = slice(s * SUPER + sub * 512, s * SUPER + (sub + 1) * 512)
                    nc.tensor.matmul(ps[:, sub * 512:(sub + 1) * 512],
                                     lhs, xts[:, 0:2, cols],
                                     start=True, stop=True,
                                     perf_mode=mybir.MatmulPerfMode.DoubleRow)
            elif _os.environ.get("KERNEL_K_OUTER", "0") == "1":
                lhs0 = xt0[:, m * P:(m + 1) * P]
                lhs1 = xt1[:, m * P:(m + 1) * P]
                # one weight load serves 4 column slices
                for k, (lhs, xt) in enumerate(((lhs0, xt0), (lhs1, xt1))):
                    for sub in range(SUB):
                        cols = slice(s * SUPER + sub * 512,
                                     s * SUPER + (sub + 1) * 512)
                        nc.tensor.matmul(ps[:, sub * 512:(sub + 1) * 512],
                                         lhs, xt[:, cols],
                                         start=(k == 0), stop=(k == 1))
            else:
                lhs0 = xt0[:, m * P:(m + 1) * P]
                lhs1 = xt1[:, m * P:(m + 1) * P]
                for sub in range(SUB):
                    cols = slice(s * SUPER + sub * 512, s * SUPER + (sub + 1) * 512)
                    pslice = ps[:, sub * 512:(sub + 1) * 512]
                    nc.tensor.matmul(pslice, lhs0, xt0[:, cols],
                                     start=True, stop=False)
                    nc.tensor.matmul(pslice, lhs1, xt1[:, cols],
                                     start=False, stop=True)
            if _os.environ.get("KERNEL_PE_ONLY", "0") == "1":
                continue
            # exp + fused row-sum; s=0 exp tiles keep their own slots
            # because DVE reads them (diag/pair extraction).
            eo = sc.tile([P, SUPER], FP32 if EO_DT == "fp32" else mybir.dt.bfloat16,
                         tag="expout0" if s == 0 else "expout")
            nc.scalar.activation(
                out=eo, in_=ps, func=mybir.ActivationFunctionType.Exp,
                scale=1.0 / ((FP8_SCALE * FP8_SCALE) if fp8 else 1.0) / TAU,
                accum_out=acc_sb[:, m * S_TILES + s:m * S_TILES + s + 1])
            if s == 0:
                gblk = eo[:, m * P:(m + 1) * P]
                tmp = sc.tile([P, P], FP32, tag="gtmp")
                nc.vector.tensor_tensor(
                    out=tmp, in0=gblk, in1=eye, op=mybir.AluOpType.mult)
                nc.vector.reduce_sum(
                    dv_sb[:, m:m + 1], tmp, axis=mybir.AxisListType.X)
                tmp2 = sc.tile([P, P], FP32, tag="gtmp")
                nc.vector.tensor_tensor(
                    out=tmp2, in0=gblk, in1=pm, op=mybir.AluOpType.mult)
                nc.vector.reduce_sum(
                    dv_sb[:, M_TILES + m:M_TILES + m + 1], tmp2,
                    axis=mybir.AxisListType.X)
                # observer: let ACT see the DVE sem so the next
                # s=0 exp's buffer WAR needs no extra wait
                obs = small.tile([P, 1], FP32, tag=f"obs{m}")
                nc.scalar.copy(out=obs,
                               in_=dv_sb[:, M_TILES + m:M_TILES + m + 1])


def _split_multi_waits(nc):
    """walrus codegen accepts at most ONE semaphore wait per engine
    instruction; Tile's wait assignment can bake in several.  Hoist all but
    the last wait of each engine instruction into standalone
    InstEventSemaphore sequencer ops right before it (the same mechanism
    barriers use) — semantics are identical, the engine blocks on the waits
    in order."""
    n_split = 0
    for blk in nc.m.functions[0].blocks:
        new_insts = []
        for inst in blk.instructions:
            si = inst.sync_info
            tname = type(inst).__name__
            if si is not None and len(si.on_wait) > 1 and tname != "InstEventSemaphore":
                waits = list(si.on_wait)
                for j, w in enumerate(waits[:-1]):
                    es = mybir.InstEventSemaphore(
                        name=f"W-split-{inst.name}-{j}")
                    es.engine = inst.engine
                    es.sync_info = mybir.SyncInfo(on_wait=[w], on_update=[])
                    new_insts.append(es)
                    nc.register_instruction(es)
                    n_split += 1
                inst.sync_info = mybir.SyncInfo(
                    on_wait=[waits[-1]], on_update=list(si.on_update))
            new_insts.append(inst)
        blk.instructions[:] = new_insts
    return n_split


def _masks():
    if EO_DT == "fp32":
        mdt = np.float32
    else:
        import ml_dtypes
        mdt = ml_dtypes.bfloat16
    eye = np.eye(P, dtype=mdt)
    pm = np.zeros((P, P), dtype=mdt)
    idx = np.arange(P)
    pm[idx, idx ^ 1] = mdt(1.0)
    return eye, pm


def _prepare_inputs(x):
    x = np.ascontiguousarray(np.asarray(x, dtype=np.float32))
    inv = 1.0 / np.sqrt((x * x).sum(axis=1))
    xn = x * inv[:, None].astype(np.float32)
    if MM_DT == "fp8":
        import ml_dtypes
        # scale up before e4m3 rounding so typical entries (~0.06) land in
        # the normal range; the ACT exp scale divides the product back out
        xnT = np.ascontiguousarray(
            (xn.T * FP8_SCALE).astype(ml_dtypes.float8_e4m3))
    elif MM_DT == "bf16":
        import ml_dtypes
        xnT = np.ascontiguousarray(xn.T.astype(ml_dtypes.bfloat16))
    else:
        xnT = np.ascontiguousarray(xn.T.astype(np.float32))  # [256, 8192]
    eye, pm = _masks()
    in_maps = []
    for c in range(NCORES):
        rolled = np.roll(xnT, -c * ROWS_PER_CORE, axis=1)
        xt = np.ascontiguousarray(rolled.reshape(2, P, N))
        in_maps.append({"xt": xt, "eye": eye, "pm": pm})
    return in_maps


def _combine(results):
    total = 0.0
    for c in range(NCORES):
        acc = np.asarray(results[c]["acc"], dtype=np.float64)   # [128, 32]
        dv = np.asarray(results[c]["dv"], dtype=np.float64)     # [128, 16]
        rowsum = acc.reshape(P, M_TILES, S_TILES).sum(axis=2)   # [p, m]
        diag_exp = dv[:, :M_TILES]                              # exp(s_ii/tau)
        v_exp = dv[:, M_TILES:]                                 # exp(v_i/tau)
        d = rowsum - diag_exp + 1.0
        total += (np.log(d) - np.log(v_exp)).sum()
    return np.float32(total / N)


def kernel(x, repeat=None):
    if repeat is None:
        repeat = int(os.environ.get("KERNEL_REPEAT", "1"))
    key = f"nc{repeat}"
    if key not in _CACHE:
        _CACHE[key] = build_nc(repeat)
    nc = _CACHE[key]
    in_maps = _prepare_inputs(x)
    trace = bool(int(os.environ.get("KERNEL_TRACE", "0")))
    res = run_bass_kernel_spmd(nc, in_maps, list(range(NCORES)), trace=trace)
    _CACHE["last_results"] = res
    return _combine(res.results)

